# revision 1
# baseline (speedup 1.0000x reference)
"""Trainium2 Bass kernel for nn_ExperimentalGNN (8-layer edge-featured GAT).

Self-contained: host-side index prep + bass program + SPMD runner over 8 cores.

Sharding: destination-partitioned graph parallelism. Each core owns 1280 node
slots (10 dst-tiles x 128). Per layer a per-node record
R = [xh = h @ Wl[l] (512) | s_src (8) | s_dst (8) | pad] is AllGathered across
cores; each core dma_gathers the records of its edges' source nodes, computes
the segment softmax with one-hot selection matmuls on the tensor engine
(p-sums + expansion), aggregates sum_e p*xh[src] as a selection matmul, and
normalizes after aggregation. Residual + LayerNorm stay core-local (h kept in
SBUF). Edge scores s_e (all 8 layers) are precomputed once on device.
"""
import os
import sys
import numpy as np

sys.path.insert(0, "/opt/trn_rl_repo")

GNN_LAYERS = int(os.environ.get("GNN_LAYERS", "8"))
GNN_SKIP_EDGE = os.environ.get("GNN_SKIP_EDGE", "0") == "1"
GNN_SKIP_SE = os.environ.get("GNN_SKIP_SE", "0") == "1"
GNN_SKIP_SEG = os.environ.get("GNN_SKIP_SEG", "0") == "1"

N = 10000
E = 160000
G = 64
D = 512
H = 8
C = 64
L = 8
DE = 256
NCORES = 8
TPC = 10                 # dst-tiles per core
NPC = TPC * 128          # node slots per core
N_PAD = NCORES * NPC
T_FIX = 18               # edge-tiles per dst-tile (final grid)
GH = 9                   # edge-tiles per gather call (half a dst-tile)
SPT = T_FIX * 128        # edge slots per dst-tile
ET = TPC * T_FIX
SE_T = 17                # edge-tiles per dst-tile in encoder grid
SE_SPT = SE_T * 128
SE_ROWS = TPC * SE_SPT
REC = 576
NEG = 0.2
MASKV = -1e30

_CACHE = {}


# ---------------- host-side prep (integer index work only) ----------------
def _host_prep(edge_index, batch):
    src0 = edge_index[0].astype(np.int64)
    dst0 = edge_index[1].astype(np.int64)
    cnt = np.bincount(dst0, minlength=N)
    inv_cnt = (1.0 / np.maximum(cnt, 1)).astype(np.float32)

    perm_slot = np.full(N, -1, np.int64)
    tile_members = {}
    for core in range(NCORES):
        nodes = np.arange(core * 1250, (core + 1) * 1250)
        deg = cnt[nodes] + 1
        order = np.argsort(-deg, kind="stable")
        loads = np.zeros(TPC, np.int64)
        fill = np.zeros(TPC, np.int64)
        assign = np.empty(1250, np.int64)
        big = np.iinfo(np.int64).max
        for idx in order:
            t = int(np.argmin(np.where(fill < 128, loads, big)))
            assign[idx] = t
            loads[t] += deg[idx]
            fill[t] += 1
        for t in range(TPC):
            members = nodes[assign == t]
            tile_members[(core, t)] = members
            base = core * NPC + t * 128
            perm_slot[members] = base + np.arange(len(members))
        assert loads.max() <= SPT, loads.max()

    slot_node = np.full(N_PAD, -1, np.int64)
    slot_node[perm_slot] = np.arange(N)
    slot_graph = np.full(N_PAD, 999, np.int64)
    valid = slot_node >= 0
    slot_graph[valid] = batch[slot_node[valid]]

    dst_slot_all = perm_slot[dst0]
    dst_core = dst_slot_all // NPC
    dst_tile = (dst_slot_all % NPC) // 128

    ng = NCORES * TPC * SPT
    g_src = np.zeros(ng, np.int64)
    g_dl = np.full(ng, 999, np.int64)
    g_mask = np.full(ng, MASKV, np.float32)
    g_se = np.zeros(ng, np.int64)
    g_isreal = np.zeros(ng, np.float32)
    g_loopdst = np.full(ng, 999, np.int64)

    se_dl = np.full((NCORES, SE_ROWS), 999, np.int64)
    se_edge = np.full((NCORES, SE_ROWS), -1, np.int64)

    for core in range(NCORES):
        for t in range(TPC):
            sel = np.where((dst_core == core) & (dst_tile == t))[0]
            order = np.argsort(dst_slot_all[sel], kind="stable")
            sel = sel[order]
            n_real = len(sel)
            members = tile_members[(core, t)]
            n_loop = len(members)
            assert n_real <= SE_SPT, n_real
            assert n_real + n_loop <= SPT

            base = (core * TPC + t) * SPT
            g_src[base:base + n_real] = perm_slot[src0[sel]]
            g_dl[base:base + n_real] = dst_slot_all[sel] % 128
            g_mask[base:base + n_real] = 0.0
            g_se[base:base + n_real] = t * SE_SPT + np.arange(n_real)
            g_isreal[base:base + n_real] = 1.0
            lo = base + n_real
            ms = perm_slot[members]
            g_src[lo:lo + n_loop] = ms
            g_dl[lo:lo + n_loop] = ms % 128
            g_mask[lo:lo + n_loop] = 0.0
            g_loopdst[lo:lo + n_loop] = ms % 128

            sbase = t * SE_SPT
            se_dl[core, sbase:sbase + n_real] = dst_slot_all[sel] % 128
            se_edge[core, sbase:sbase + n_real] = sel

    return dict(perm_slot=perm_slot, slot_node=slot_node, slot_graph=slot_graph,
                inv_cnt=inv_cnt, g_src=g_src, g_dl=g_dl, g_mask=g_mask,
                g_se=g_se, g_isreal=g_isreal, g_loopdst=g_loopdst,
                se_dl=se_dl, se_edge=se_edge)


def _wrap_idx(flat):
    n = len(flat)
    w = np.asarray(flat, np.int16).reshape(n // 16, 16).T
    return np.tile(w, (8, 1))


def _grid_cols(arr, dtype):
    a = np.asarray(arr).reshape(-1, 128).T
    return np.ascontiguousarray(a).astype(dtype)


def _build_inputs(inp):
    edge_index = np.asarray(inp["edge_index"])
    batch = np.asarray(inp["batch"])
    prep = _host_prep(edge_index, batch)

    x = np.asarray(inp["x"], np.float32)
    ef = np.asarray(inp["edge_attr"], np.float32)[:, 1:3]
    x_perm = np.zeros((N_PAD, 4), np.float32)
    x_perm[prep["perm_slot"]] = x

    At_src = np.zeros((L, D, H), np.float32)
    At_dst = np.zeros((L, D, H), np.float32)
    At_e = np.zeros((L, D, H), np.float32)
    for l in range(L):
        for h in range(H):
            At_src[l, h * C:(h + 1) * C, h] = np.asarray(inp["a_src"])[l, h]
            At_dst[l, h * C:(h + 1) * C, h] = np.asarray(inp["a_dst"])[l, h]
            At_e[l, h * C:(h + 1) * C, h] = np.asarray(inp["a_e"])[l, h]
    At_sd = np.concatenate([At_src, At_dst], axis=2)

    Wl = np.asarray(inp["Wl"], np.float32)
    WlT = np.ascontiguousarray(np.transpose(Wl, (0, 2, 1)))
    WleT = np.ascontiguousarray(np.transpose(np.asarray(inp["Wle"], np.float32),
                                             (0, 2, 1)))

    shared = {
        "wn": np.asarray(inp["Wn"], np.float32),
        "bn": np.asarray(inp["bn"], np.float32)[None, :],
        "wee": np.asarray(inp["Wee"], np.float32),
        "bee": np.asarray(inp["bee"], np.float32)[None, :],
        "wleT": WleT.reshape(L * D, DE),
        "at_e": At_e.reshape(L * D, H),
        "at_sd": At_sd.reshape(L * D, 16),
        "wl": Wl.reshape(L * D, D),
        "wlT": WlT.reshape(L * D, D),
        "bl": np.asarray(inp["bl"], np.float32),
        "wp": np.asarray(inp["Wp"], np.float32),
        "bp": np.asarray(inp["bp"], np.float32)[None, :],
        "wg1": np.asarray(inp["Wg1"], np.float32),
        "bg1": np.asarray(inp["bg1"], np.float32)[None, :],
        "wg2": np.asarray(inp["Wg2"], np.float32),
        "bg2": np.asarray(inp["bg2"], np.float32)[None, :],
        "wf1": np.asarray(inp["Wf1"], np.float32),
        "bf1": np.asarray(inp["bf1"], np.float32)[None, :],
        "wf2": np.asarray(inp["Wf2"], np.float32),
        "bf2": np.asarray(inp["bf2"], np.float32)[None, :],
        "nAT": np.asarray(inp["nA"], np.float32),
        "nBT": np.asarray(inp["nB"], np.float32),
        "sysT": np.asarray(inp["system_size"], np.float32),
    }

    in_maps = []
    for core in range(NCORES):
        lo = core * NPC
        gsl = slice(core * TPC * SPT, (core + 1) * TPC * SPT)
        gidx = np.concatenate(
            [_wrap_idx(prep["g_src"][gsl][i * GH * 128:(i + 1) * GH * 128])
             for i in range(TPC * 2)], axis=1)
        seidx = np.concatenate(
            [_wrap_idx(prep["g_se"][gsl][t * SPT:(t + 1) * SPT])
             for t in range(TPC)], axis=1)
        efc = np.zeros((SE_ROWS, 2), np.float32)
        rows = prep["se_edge"][core]
        v = rows >= 0
        efc[v] = ef[rows[v]]
        ic = np.zeros((128, TPC), np.float32)
        gid = np.full((128, TPC), 999.0, np.float32)
        for t in range(TPC):
            slots = lo + t * 128 + np.arange(128)
            nodes = prep["slot_node"][slots]
            ok = nodes >= 0
            ic[ok, t] = prep["inv_cnt"][nodes[ok]]
            gid[:, t] = prep["slot_graph"][slots]
        m = dict(shared)
        m.update({
            "xT": np.ascontiguousarray(x_perm[lo:lo + NPC].T),
            "efT": np.ascontiguousarray(efc.T),
            "gidx": gidx.astype(np.int16),
            "seidx": seidx.astype(np.int16),
            "dstloc": _grid_cols(prep["g_dl"][gsl], np.float32),
            "emask": _grid_cols(prep["g_mask"][gsl], np.float32),
            "isreal": _grid_cols(prep["g_isreal"][gsl], np.float32),
            "loopdst": _grid_cols(prep["g_loopdst"][gsl], np.float32),
            "sedl": _grid_cols(prep["se_dl"][core], np.float32),
            "invcnt": ic,
            "graphid": gid,
        })
        in_maps.append(m)
    return in_maps


# ---------------- bass program ----------------
def _build_program():
    import contextlib
    import concourse.bass as bass
    import concourse.bacc as bacc
    import concourse.tile as tile
    import concourse.mybir as mybir
    from concourse.masks import make_identity

    dt = mybir.dt
    AF = mybir.ActivationFunctionType
    OP = mybir.AluOpType

    nc = bacc.Bacc("TRN2", target_bir_lowering=False, debug=False,
                   num_devices=NCORES)

    def din(name, shape, dtype=dt.float32):
        return nc.dram_tensor(name, shape, dtype, kind="ExternalInput")

    xT = din("xT", [4, NPC])
    efT = din("efT", [2, SE_ROWS])
    gidx = din("gidx", [128, TPC * SPT // 16], dt.int16)
    seidx = din("seidx", [128, TPC * SPT // 16], dt.int16)
    dstloc = din("dstloc", [128, ET])
    emask = din("emask", [128, ET])
    isreal = din("isreal", [128, ET])
    loopdst = din("loopdst", [128, ET])
    sedl = din("sedl", [128, TPC * SE_T])
    invcnt = din("invcnt", [128, TPC])
    graphid = din("graphid", [128, TPC])
    wn = din("wn", [4, D]); bn = din("bn", [1, D])
    wee = din("wee", [2, DE]); bee = din("bee", [1, DE])
    wleT = din("wleT", [L * D, DE])
    at_e = din("at_e", [L * D, H])
    at_sd = din("at_sd", [L * D, 16])
    wl = din("wl", [L * D, D])
    wlT = din("wlT", [L * D, D])
    bl = din("bl", [L, D])
    wp = din("wp", [D, D]); bp = din("bp", [1, D])
    wg1 = din("wg1", [2, DE]); bg1 = din("bg1", [1, DE])
    wg2 = din("wg2", [DE, DE]); bg2 = din("bg2", [1, DE])
    wf1 = din("wf1", [D + DE, DE]); bf1 = din("bf1", [1, DE])
    wf2 = din("wf2", [DE, 1]); bf2 = din("bf2", [1, 1])
    nAT = din("nAT", [G, 1]); nBT = din("nBT", [G, 1]); sysT = din("sysT", [G, 1])

    out_t = nc.dram_tensor("out", [G, 1], dt.float32, kind="ExternalOutput")
    RG = [list(range(NCORES))]

    with tile.TileContext(nc) as tc:
        stack = contextlib.ExitStack()
        cst = stack.enter_context(tc.tile_pool(name="cst", bufs=1))
        res = stack.enter_context(tc.tile_pool(name="res", bufs=1))
        wk = stack.enter_context(tc.tile_pool(name="wk", bufs=2))
        gat = stack.enter_context(tc.tile_pool(name="gat", bufs=2))
        ps = stack.enter_context(tc.tile_pool(name="ps", bufs=1, space="PSUM"))
        dram = stack.enter_context(tc.tile_pool(name="dram", bufs=1, space="DRAM"))

        R_bufs = [dram.tile([N_PAD, REC], dt.float32, addr_space="Shared",
                            name=f"R_{i}") for i in range(L)]
        ag_in = dram.tile([NPC, REC], dt.float32)
        se_all = dram.tile([SE_ROWS, 64], dt.float32)
        pool_in = dram.tile([G, D], dt.float32)
        pool_out = dram.tile([G, D], dt.float32, addr_space="Shared")

        def load_kxn(rows_ap, nchunk, ncols, name, pool=wk, tag=None):
            t = pool.tile([128, nchunk * ncols], dt.float32, name=name,
                          tag=tag or name)
            for kc in range(nchunk):
                nc.sync.dma_start(
                    out=t[:, kc * ncols:(kc + 1) * ncols],
                    in_=rows_ap[kc * 128:(kc + 1) * 128, :])
            return t

        def ldma(src_ap, shape, name, pool=cst, dtype=dt.float32, tag=None,
                 bufs=None):
            t = pool.tile(list(shape), dtype, name=name, tag=tag or name,
                          bufs=bufs)
            nc.sync.dma_start(out=t[:], in_=src_ap)
            return t

        def rep_row(row_ap, p, f, name, pool=cst, tag=None, bufs=None):
            t = pool.tile([p, f], dt.float32, name=name, tag=tag or name,
                          bufs=bufs)
            nc.sync.dma_start(out=t[:], in_=row_ap.to_broadcast((p, f)))
            return t

        # constants
        ident_g = cst.tile([128, 128], dt.float32)
        make_identity(nc, ident_g[:])
        ident = cst.tile([128, 128], dt.float32)
        nc.vector.tensor_copy(ident[:], ident_g[:])
        iota_i = cst.tile([128, 128], dt.int32)
        nc.gpsimd.iota(iota_i[:], pattern=[[1, 128]], base=0, channel_multiplier=0)
        iotaF = cst.tile([128, 128], dt.float32)
        nc.vector.tensor_copy(iotaF[:], iota_i[:])
        iota64_i = cst.tile([128, G], dt.int32)
        nc.gpsimd.iota(iota64_i[:], pattern=[[1, G]], base=0, channel_multiplier=0)
        iota64 = cst.tile([128, G], dt.float32)
        nc.vector.tensor_copy(iota64[:], iota64_i[:])

        wn_sb = ldma(wn[:], (4, D), "wn_sb")
        wee_sb = ldma(wee[:], (2, DE), "wee_sb")
        dstloc_sb = ldma(dstloc[:], (128, ET), "dstloc_sb")
        emask_sb = ldma(emask[:], (128, ET), "emask_sb")
        isreal_sb = ldma(isreal[:], (128, ET), "isreal_sb")
        loopdst_sb = ldma(loopdst[:], (128, ET), "loopdst_sb")
        sedl_sb = ldma(sedl[:], (128, TPC * SE_T), "sedl_sb")
        invcnt_sb = ldma(invcnt[:], (128, TPC), "invcnt_sb")
        graphid_sb = ldma(graphid[:], (128, TPC), "graphid_sb")
        gidx_sb = ldma(gidx[:], (128, TPC * SPT // 16), "gidx_sb", dtype=dt.int16)
        seidx_sb = ldma(seidx[:], (128, TPC * SPT // 16), "seidx_sb",
                        dtype=dt.int16)
        bn_rep = rep_row(bn[:], 128, D, "bn_rep")
        bee_rep = rep_row(bee[:], 128, DE, "bee_rep")
        xT_sb = ldma(xT[:], (4, NPC), "xT_sb")

        h_my = res.tile([128, TPC * D], dt.float32)
        s_my = res.tile([128, TPC * 16], dt.float32)
        s_e_sb = res.tile([128, ET * 64], dt.float16)
        loopse = res.tile([128, TPC * 64], dt.float32)

        def psum(shape, tag, bufs, name):
            return ps.tile(list(shape), dt.float32, space="PSUM", name=name,
                           tag=tag, bufs=bufs)

        def ln_store(src_ap, dst_ap, F, bias_rep=None, do_elu=True,
                     residual_ap=None, P=128):
            x1 = wk.tile([P, F], dt.float32, name="ln_x1", tag="ln_x1")
            if bias_rep is not None:
                nc.vector.tensor_tensor(out=x1[:], in0=src_ap,
                                        in1=bias_rep[:P, :F], op=OP.add)
            else:
                nc.vector.tensor_copy(x1[:], src_ap)
            sums = wk.tile([P, 1], dt.float32, name="ln_sum", tag="ln_sum")
            sc = wk.tile([P, F], dt.float32, name="ln_sc", tag="ln_sc")
            nc.scalar.activation(out=sc[:], in_=x1[:], func=AF.Copy,
                                 accum_out=sums[:])
            mean = wk.tile([P, 1], dt.float32, name="ln_mean", tag="ln_mean")
            nc.vector.tensor_scalar_mul(mean[:], sums[:], 1.0 / F)
            nc.vector.tensor_scalar(out=x1[:], in0=x1[:], scalar1=mean[:],
                                    scalar2=None, op0=OP.subtract)
            sq = wk.tile([P, 1], dt.float32, name="ln_sq", tag="ln_sq")
            nc.scalar.activation(out=sc[:], in_=x1[:], func=AF.Square,
                                 accum_out=sq[:])
            rstd = wk.tile([P, 1], dt.float32, name="ln_rstd", tag="ln_rstd")
            nc.vector.tensor_scalar(out=rstd[:], in0=sq[:], scalar1=1.0 / F,
                                    scalar2=1e-5, op0=OP.mult, op1=OP.add)
            nc.scalar.activation(out=rstd[:], in_=rstd[:], func=AF.Sqrt)
            nc.vector.reciprocal(rstd[:], rstd[:])
            nc.vector.tensor_scalar(out=x1[:], in0=x1[:], scalar1=rstd[:],
                                    scalar2=None, op0=OP.mult)
            if do_elu:
                tmin = wk.tile([P, F], dt.float32, name="ln_tm", tag="ln_tm")
                nc.vector.tensor_scalar_min(tmin[:], x1[:], 0.0)
                nc.scalar.activation(out=tmin[:], in_=tmin[:], func=AF.Exp)
                nc.vector.tensor_scalar_max(x1[:], x1[:], 0.0)
                nc.vector.tensor_tensor(out=x1[:], in0=x1[:], in1=tmin[:],
                                        op=OP.add)
                nc.vector.tensor_scalar_add(x1[:], x1[:], -1.0)
            if residual_ap is not None:
                nc.vector.tensor_tensor(out=dst_ap, in0=x1[:], in1=residual_ap,
                                        op=OP.add)
            else:
                nc.vector.tensor_copy(dst_ap, x1[:])

        def transpose_chunks(src_ap, nchunk, rows=128):
            dst = wk.tile([128, nchunk * rows], dt.float32, name="trT", tag="trT")
            for ci in range(nchunk):
                pt = psum([128, rows], "tr", 2, "tr_ps")
                nc.tensor.transpose(out=pt[:],
                                    in_=src_ap[:, ci * 128:(ci + 1) * 128],
                                    identity=ident[:rows, :rows])
                nc.vector.tensor_copy(dst[:, ci * rows:(ci + 1) * rows], pt[:])
            return dst

        # ---------- setup: Wes / Wsd ----------
        wes_sb = cst.tile([128, 2 * 64], dt.float32)
        for l in range(L):
            for m in range(2):
                pt = psum([128, H], "e8", 1, "wes_ps")
                for k in range(4):
                    lhs = ldma(wleT[l * D + k * 128:l * D + (k + 1) * 128,
                                    m * 128:(m + 1) * 128], (128, 128), "wleT_c",
                               pool=wk, tag="wleT_c")
                    rhs = ldma(at_e[l * D + k * 128:l * D + (k + 1) * 128, :],
                               (128, H), "ate_c", pool=wk, tag="ate_c")
                    nc.tensor.matmul(pt[:], lhs[:], rhs[:], start=(k == 0),
                                     stop=(k == 3))
                nc.vector.tensor_copy(
                    wes_sb[:, m * 64 + l * 8:m * 64 + (l + 1) * 8], pt[:])

        wsd_sb = cst.tile([128, 4 * L * 16], dt.float32)
        for l in range(L):
            for kc in range(4):
                pt = psum([128, 16], "tr", 2, "wsd_ps")
                for oc in range(4):
                    lhs = ldma(wlT[l * D + oc * 128:l * D + (oc + 1) * 128,
                                   kc * 128:(kc + 1) * 128], (128, 128), "wlT_c",
                               pool=wk, tag="wleT_c")
                    rhs = ldma(at_sd[l * D + oc * 128:l * D + (oc + 1) * 128, :],
                               (128, 16), "atsd_c", pool=wk, tag="ate_c")
                    nc.tensor.matmul(pt[:], lhs[:], rhs[:], start=(oc == 0),
                                     stop=(oc == 3))
                nc.vector.tensor_copy(
                    wsd_sb[:, (kc * L + l) * 16:(kc * L + l + 1) * 16], pt[:])

        # ---------- setup: h0 ----------
        for t in range(TPC):
            pt = psum([128, D], "gemm", 2, "h0_ps")
            nc.tensor.matmul(pt[:], xT_sb[:, t * 128:(t + 1) * 128], wn_sb[:],
                             start=True, stop=True)
            ln_store(pt[:], h_my[:, t * D:(t + 1) * D], D, bias_rep=bn_rep,
                     do_elu=True)

        # ---------- setup: s_e encoder over real-edge grid ----------
        if GNN_SKIP_SE:
            nc.vector.memset(s_e_sb[:], 0.0)
        for t in range(TPC if not GNN_SKIP_SE else 0):
            lps = psum([128, 64], "sacc", 1, "loop_ps")
            for k in range(SE_T):
                i = t * SE_T + k
                eft = ldma(efT[:, i * 128:(i + 1) * 128], (2, 128), "eft",
                           pool=wk, tag="eft")
                ept = psum([128, DE], "gemm", 2, "ee_ps")
                nc.tensor.matmul(ept[:], eft[:], wee_sb[:],
                                 start=True, stop=True)
                ee = wk.tile([128, DE], dt.float32, name="ee_sb", tag="ee_sb")
                ln_store(ept[:], ee[:], DE, bias_rep=bee_rep, do_elu=True)
                eeT = transpose_chunks(ee[:], 2)
                spt = psum([128, 64], "e8", 1, "se_ps")
                for mc in range(2):
                    nc.tensor.matmul(spt[:], eeT[:, mc * 128:(mc + 1) * 128],
                                     wes_sb[:, mc * 64:(mc + 1) * 64],
                                     start=(mc == 0), stop=(mc == 1))
                sev = wk.tile([128, 64], dt.float32, name="sev", tag="sev")
                nc.vector.tensor_copy(sev[:], spt[:])
                nc.sync.dma_start(out=se_all[i * 128:(i + 1) * 128, :], in_=sev[:])
                selT = wk.tile([128, 128], dt.float32, name="selTse", tag="selT")
                nc.vector.tensor_tensor(
                    out=selT[:], in0=sedl_sb[:, i:i + 1].to_broadcast((128, 128)),
                    in1=iotaF[:], op=OP.is_equal)
                nc.tensor.matmul(lps[:], selT[:], sev[:], start=(k == 0),
                                 stop=(k == SE_T - 1))
            nc.vector.tensor_tensor(
                out=loopse[:, t * 64:(t + 1) * 64], in0=lps[:],
                in1=invcnt_sb[:, t:t + 1].to_broadcast((128, 64)), op=OP.mult)

        # ---------- setup: gather s_e into final grid + loop fixup ----------
        if GNN_SKIP_SEG and not GNN_SKIP_SE:
            nc.vector.memset(s_e_sb[:], 0.0)
        for t in range(TPC if not (GNN_SKIP_SE or GNN_SKIP_SEG) else 0):
            for hf in range(2):
                sg = gat.tile([128, GH * 64], dt.float32, name="seg", tag="gt")
                nc.gpsimd.dma_gather(
                    out_ap=sg[:].rearrange("p (t e) -> p t e", e=64),
                    in_ap=se_all[:],
                    idxs_ap=seidx_sb[:, (t * 2 + hf) * (GH * 8):
                                     (t * 2 + hf + 1) * (GH * 8)],
                    num_idxs=GH * 128, num_idxs_reg=GH * 128, elem_size=64,
                    single_packet=False)
                nc.vector.tensor_tensor(
                    out=sg[:].rearrange("p (t e) -> p t e", e=64),
                    in0=sg[:].rearrange("p (t e) -> p t e", e=64),
                    in1=isreal_sb[:, t * T_FIX + hf * GH:t * T_FIX + (hf + 1) * GH]
                    [:, :, None].to_broadcast((128, GH, 64)),
                    op=OP.mult)
                for jj in range(GH):
                    j = hf * GH + jj
                    tj = t * T_FIX + j
                    selT = wk.tile([128, 128], dt.float32, name="selTlp",
                                   tag="selT")
                    nc.vector.tensor_tensor(
                        out=selT[:],
                        in0=loopdst_sb[:, tj:tj + 1].to_broadcast((128, 128)),
                        in1=iotaF[:], op=OP.is_equal)
                    lpt = psum([128, 128], "tr", 2, "sellp_ps")
                    nc.tensor.transpose(out=lpt[:], in_=selT[:], identity=ident[:])
                    sel_lp = wk.tile([128, 128], dt.float32, name="sel_lp",
                                     tag="sel_sb")
                    nc.vector.tensor_copy(sel_lp[:], lpt[:])
                    ept = psum([128, 64], "e8", 1, "lpx_ps")
                    nc.tensor.matmul(ept[:], sel_lp[:],
                                     loopse[:, t * 64:(t + 1) * 64],
                                     start=True, stop=True)
                    nc.vector.tensor_tensor(out=sg[:, jj * 64:(jj + 1) * 64],
                                            in0=sg[:, jj * 64:(jj + 1) * 64],
                                            in1=ept[:], op=OP.add)
                nc.vector.tensor_copy(
                    s_e_sb[:, (t * T_FIX + hf * GH) * 64:
                           (t * T_FIX + (hf + 1) * GH) * 64], sg[:])

        # ---------- record emit (xh | s) for layer lx from current h ----------
        def emit_record(t, lx, wl_tile):
            hT = transpose_chunks(h_my[:, t * D:(t + 1) * D], 4)
            xp = psum([128, D], "gemm", 2, "xh_ps")
            for k in range(4):
                nc.tensor.matmul(xp[:], hT[:, k * 128:(k + 1) * 128],
                                 wl_tile[:, k * D:(k + 1) * D],
                                 start=(k == 0), stop=(k == 3))
            xh_sb = wk.tile([128, D], dt.float32, name="xh_sb", tag="xh_sb")
            nc.vector.tensor_copy(xh_sb[:], xp[:])
            s16 = psum([128, 16], "tr", 2, "s16_ps")
            for k in range(4):
                nc.tensor.matmul(s16[:], hT[:, k * 128:(k + 1) * 128],
                                 wsd_sb[:, (k * L + lx) * 16:(k * L + lx + 1) * 16],
                                 start=(k == 0), stop=(k == 3))
            nc.vector.tensor_copy(s_my[:, t * 16:(t + 1) * 16], s16[:])
            nc.sync.dma_start(out=ag_in[t * 128:(t + 1) * 128, 0:D], in_=xh_sb[:])
            nc.sync.dma_start(out=ag_in[t * 128:(t + 1) * 128, D:D + 16],
                              in_=s_my[:, t * 16:(t + 1) * 16])

        wl_cur = load_kxn(wl[0:D, :], 4, D, "wl_sb0", tag="wl_sb")
        for t in range(TPC):
            emit_record(t, 0, wl_cur)
        nc.gpsimd.collective_compute("AllGather", OP.bypass, replica_groups=RG,
                                     ins=[ag_in.opt()], outs=[R_bufs[0].opt()])

        # ---------- layers ----------
        pool_holder = [None]
        LL = GNN_LAYERS
        for l in range(LL):
            R_cur = R_bufs[l]
            R_next = R_bufs[l + 1] if l < L - 1 else None
            wl_next = None
            if l < L - 1:
                wl_next = load_kxn(wl[(l + 1) * D:(l + 2) * D, :], 4, D,
                                   f"wl_sb{l + 1}", tag="wl_sb")
            bl_rep = rep_row(bl[l:l + 1, :], 128, D, f"bl_rep{l}", pool=wk,
                             tag="bl_rep")
            for t in range(TPC):
                s_ps = psum([128, H], "sacc", 1, "s_ps")
                agg_ps = psum([128, D], "agg", 2, "agg_ps")
                pb = wk.tile([128, T_FIX * H], dt.float32, name="pb", tag="pb")
                if GNN_SKIP_EDGE:
                    nc.tensor.matmul(s_ps[:], ident[:], ident[:, :H],
                                     start=True, stop=True)
                    nc.tensor.matmul(agg_ps[:], ident[:], h_my[:, :D],
                                     start=True, stop=True)
                for hf in range(2 if not GNN_SKIP_EDGE else 0):
                    gt = gat.tile([128, GH * REC], dt.float32, name="gt", tag="gt")
                    nc.gpsimd.dma_gather(
                        out_ap=gt[:].rearrange("p (t e) -> p t e", e=REC),
                        in_ap=R_cur[:],
                        idxs_ap=gidx_sb[:, (t * 2 + hf) * (GH * 8):
                                        (t * 2 + hf + 1) * (GH * 8)],
                        num_idxs=GH * 128, num_idxs_reg=GH * 128, elem_size=REC,
                        single_packet=False)
                    for jj in range(GH):
                        j = hf * GH + jj
                        tj = t * T_FIX + j
                        selT = wk.tile([128, 128], dt.float32, name="selT",
                                       tag="selT")
                        nc.vector.tensor_tensor(
                            out=selT[:],
                            in0=dstloc_sb[:, tj:tj + 1].to_broadcast((128, 128)),
                            in1=iotaF[:], op=OP.is_equal)
                        tps = psum([128, 128], "tr", 2, "selb_ps")
                        nc.tensor.transpose(out=tps[:], in_=selT[:],
                                            identity=ident[:])
                        sel_sb = wk.tile([128, 128], dt.float32, name="sel_sb",
                                         tag="sel_sb")
                        nc.vector.tensor_copy(sel_sb[:], tps[:])
                        eps8 = psum([128, H], "e8", 1, "eps8")
                        nc.tensor.matmul(eps8[:], sel_sb[:],
                                         s_my[:, t * 16 + 8:(t + 1) * 16],
                                         start=True, stop=True)
                        se32 = wk.tile([128, H], dt.float32, name="se32",
                                       tag="se32")
                        nc.scalar.activation(
                            out=se32[:],
                            in_=s_e_sb[:, tj * 64 + l * 8:tj * 64 + (l + 1) * 8],
                            func=AF.Copy)
                        al = wk.tile([128, H], dt.float32, name="al", tag="al")
                        nc.vector.tensor_tensor(
                            out=al[:], in0=gt[:, jj * REC + D:jj * REC + D + 8],
                            in1=se32[:], op=OP.add)
                        nc.vector.tensor_tensor(out=al[:], in0=al[:], in1=eps8[:],
                                                op=OP.add)
                        al2 = wk.tile([128, H], dt.float32, name="al2", tag="al2")
                        nc.vector.tensor_scalar_mul(al2[:], al[:], NEG)
                        nc.vector.tensor_tensor(out=al[:], in0=al[:], in1=al2[:],
                                                op=OP.max)
                        nc.scalar.activation(out=pb[:, j * H:(j + 1) * H],
                                             in_=al[:], func=AF.Exp,
                                             bias=emask_sb[:, tj:tj + 1])
                        nc.tensor.matmul(s_ps[:], selT[:],
                                         pb[:, j * H:(j + 1) * H],
                                         start=(j == 0), stop=(j == T_FIX - 1))
                        y = wk.tile([128, D], dt.float32, name="y", tag="y")
                        nc.vector.tensor_tensor(
                            out=y[:].rearrange("p (h c) -> p h c", c=C),
                            in0=gt[:, jj * REC:jj * REC + D]
                            .rearrange("p (h c) -> p h c", c=C),
                            in1=pb[:, j * H:(j + 1) * H][:, :, None]
                            .to_broadcast((128, H, C)),
                            op=OP.mult)
                        nc.tensor.matmul(agg_ps[:], selT[:], y[:],
                                         start=(j == 0), stop=(j == T_FIX - 1))
                # ---- phase B ----
                spl = wk.tile([128, H], dt.float32, name="spl", tag="spl")
                nc.vector.tensor_scalar_add(spl[:], s_ps[:], 1e-16)
                rr = wk.tile([128, H], dt.float32, name="rr", tag="rr")
                nc.vector.reciprocal(rr[:], spl[:])
                aggn = wk.tile([128, D], dt.float32, name="aggn", tag="aggn")
                nc.vector.tensor_tensor(
                    out=aggn[:].rearrange("p (h c) -> p h c", c=C),
                    in0=agg_ps[:].rearrange("p (h c) -> p h c", c=C),
                    in1=rr[:][:, :, None].to_broadcast((128, H, C)), op=OP.mult)
                ln_store(aggn[:], h_my[:, t * D:(t + 1) * D], D, bias_rep=bl_rep,
                         do_elu=False, residual_ap=h_my[:, t * D:(t + 1) * D])
                if l == LL - 1:
                    gsel = wk.tile([128, G], dt.float32, name="gsel", tag="gsel")
                    nc.vector.tensor_tensor(
                        out=gsel[:],
                        in0=graphid_sb[:, t:t + 1].to_broadcast((128, G)),
                        in1=iota64[:], op=OP.is_equal)
                    if pool_holder[0] is None:
                        pool_holder[0] = psum([G, D], "gemm", 2, "pool_ps")
                    nc.tensor.matmul(pool_holder[0][:], gsel[:],
                                     h_my[:, t * D:(t + 1) * D],
                                     start=(t == 0), stop=(t == TPC - 1))
                else:
                    emit_record(t, l + 1, wl_next)
            if l < LL - 1:
                nc.gpsimd.collective_compute("AllGather", OP.bypass,
                                             replica_groups=RG,
                                             ins=[ag_in.opt()],
                                             outs=[R_next.opt()])

        # ---------- readout ----------
        pool_sb = wk.tile([G, D], dt.float32, name="pool_sb", tag="pool_sb")
        nc.vector.tensor_copy(pool_sb[:], pool_holder[0][:])
        nc.sync.dma_start(out=pool_in[:], in_=pool_sb[:])
        nc.gpsimd.collective_compute("AllReduce", OP.add, replica_groups=RG,
                                     ins=[pool_in.opt()], outs=[pool_out.opt()])
        hp = ldma(pool_out[:], (G, D), "hp_sb", pool=wk, tag="pool_sb")

        def transpose_to64(src_ap, nchunk):
            dst = wk.tile([128, nchunk * G], dt.float32, name="t64", tag="t64")
            for ci in range(nchunk):
                pt = psum([128, G], "tr", 2, "t64_ps")
                nc.tensor.transpose(out=pt[:],
                                    in_=src_ap[:, ci * 128:(ci + 1) * 128],
                                    identity=ident[:G, :G])
                nc.vector.tensor_copy(dst[:, ci * G:(ci + 1) * G], pt[:])
            return dst

        wp_sb = load_kxn(wp[:], 4, D, "wp_sb", tag="wl_sb")
        bp_rep = rep_row(bp[:], G, D, "bp_rep", pool=wk, tag="bl_rep")
        hpT = transpose_to64(hp[:], 4)
        hr_ps = psum([G, D], "gemm", 2, "hr_ps")
        for k in range(4):
            nc.tensor.matmul(hr_ps[:], hpT[:, k * G:(k + 1) * G],
                             wp_sb[:, k * D:(k + 1) * D], start=(k == 0),
                             stop=(k == 3))
        h_r = wk.tile([G, D], dt.float32, name="h_r", tag="h_r")
        ln_store(hr_ps[:], h_r[:], D, bias_rep=bp_rep, do_elu=True, P=G)

        nA_sb = ldma(nAT[:], (G, 1), "nA_sb")
        nB_sb = ldma(nBT[:], (G, 1), "nB_sb")
        sys_sb = ldma(sysT[:], (G, 1), "sys_sb")
        invg = wk.tile([G, 1], dt.float32, name="invg", tag="invg")
        nc.vector.tensor_scalar_add(invg[:], sys_sb[:], 1e-10)
        nc.vector.reciprocal(invg[:], invg[:])
        gf = wk.tile([G, 2], dt.float32, name="gf", tag="gf")
        nc.vector.tensor_tensor(out=gf[:, 0:1], in0=nA_sb[:], in1=invg[:],
                                op=OP.mult)
        nc.vector.tensor_tensor(out=gf[:, 1:2], in0=nB_sb[:], in1=invg[:],
                                op=OP.mult)
        gft_ps = psum([2, G], "tr", 2, "gft_ps")
        nc.tensor.transpose(out=gft_ps[:], in_=gf[:], identity=ident[:G, :G])
        gfT = wk.tile([2, G], dt.float32, name="gfT", tag="gfT")
        nc.vector.tensor_copy(gfT[:], gft_ps[:])

        wg1_sb = ldma(wg1[:], (2, DE), "wg1_sb")
        bg1_rep = rep_row(bg1[:], G, DE, "bg1_rep", pool=wk, tag="bl_rep")
        g1_ps = psum([G, DE], "gemm", 2, "g1_ps")
        nc.tensor.matmul(g1_ps[:], gfT[:], wg1_sb[:], start=True, stop=True)
        gm1 = wk.tile([G, DE], dt.float32, name="gm1", tag="gm1")
        ln_store(g1_ps[:], gm1[:], DE, bias_rep=bg1_rep, do_elu=True, P=G)

        wg2_sb = load_kxn(wg2[:], 2, DE, "wg2_sb", tag="wl_sb")
        bg2_rep = rep_row(bg2[:], G, DE, "bg2_rep", pool=wk, tag="bl_rep")
        gm1T = transpose_to64(gm1[:], 2)
        g2_ps = psum([G, DE], "gemm", 2, "g2_ps")
        for k in range(2):
            nc.tensor.matmul(g2_ps[:], gm1T[:, k * G:(k + 1) * G],
                             wg2_sb[:, k * DE:(k + 1) * DE], start=(k == 0),
                             stop=(k == 1))
        gm2 = wk.tile([G, DE], dt.float32, name="gm2", tag="gm2")
        ln_store(g2_ps[:], gm2[:], DE, bias_rep=bg2_rep, do_elu=True, P=G)

        wf1_sb = load_kxn(wf1[:], 6, DE, "wf1_sb", tag="wl_sb")
        bf1_rep = rep_row(bf1[:], G, DE, "bf1_rep", pool=wk, tag="bl_rep")
        hrT = transpose_to64(h_r[:], 4)
        gm2T = wk.tile([128, 2 * G], dt.float32, name="gm2T", tag="gm2T")
        for ci in range(2):
            pt = psum([128, G], "tr", 2, "gm2t_ps")
            nc.tensor.transpose(out=pt[:], in_=gm2[:, ci * 128:(ci + 1) * 128],
                                identity=ident[:G, :G])
            nc.vector.tensor_copy(gm2T[:, ci * G:(ci + 1) * G], pt[:])
        f1_ps = psum([G, DE], "gemm", 2, "f1_ps")
        for k in range(4):
            nc.tensor.matmul(f1_ps[:], hrT[:, k * G:(k + 1) * G],
                             wf1_sb[:, k * DE:(k + 1) * DE], start=(k == 0),
                             stop=False)
        for k in range(2):
            nc.tensor.matmul(f1_ps[:], gm2T[:, k * G:(k + 1) * G],
                             wf1_sb[:, (4 + k) * DE:(5 + k) * DE], start=False,
                             stop=(k == 1))
        f1 = wk.tile([G, DE], dt.float32, name="f1", tag="f1")
        ln_store(f1_ps[:], f1[:], DE, bias_rep=bf1_rep, do_elu=True, P=G)

        wf2_sb = load_kxn(wf2[:], 2, 1, "wf2_sb", tag="wf2_sb")
        bf2_rep = rep_row(bf2[:], G, 1, "bf2_rep", pool=wk, tag="bf2_rep")
        f1T = transpose_to64(f1[:], 2)
        o_ps = psum([G, 1], "e8", 1, "o_ps")
        for k in range(2):
            nc.tensor.matmul(o_ps[:], f1T[:, k * G:(k + 1) * G],
                             wf2_sb[:, k:k + 1], start=(k == 0), stop=(k == 1))
        ovec = wk.tile([G, 1], dt.float32, name="ovec", tag="ovec")
        nc.vector.tensor_tensor(out=ovec[:], in0=o_ps[:], in1=bf2_rep[:],
                                op=OP.add)
        nc.sync.dma_start(out=out_t[:], in_=ovec[:])

        stack.close()

    nc.compile()
    return nc


def kernel(**inputs) -> np.ndarray:
    from concourse.bass_utils import run_bass_kernel_spmd
    if "nc" not in _CACHE:
        _CACHE["nc"] = _build_program()
    nc = _CACHE["nc"]
    in_maps = _build_inputs(inputs)
    res = run_bass_kernel_spmd(nc, in_maps, core_ids=list(range(NCORES)))
    out = res.results[0]["out"]
    return np.asarray(out).reshape(G).astype(np.float32)



# revision 11
# speedup vs baseline: 4.4224x; 4.4224x over previous
"""Trainium2 Bass kernel for nn_ExperimentalGNN (8-layer edge-featured GAT).

Self-contained: host-side index prep + bass program + SPMD runner over 8 cores.

Sharding: destination-partitioned graph parallelism. Each core owns 1280 node
slots (10 dst-tiles x 128). Per layer a per-node record
R = [xh = h @ Wl[l] (512 bf16) | s_src (8 bf16) | pad] is AllGathered across
cores; each core dma_gathers the records of its edges' source nodes (bf16,
1280B each), computes the segment softmax with static selection matrices
(built once, streamed from DRAM), and aggregates sum_e p*xh[src] with bf16
selection matmuls. Residual + LayerNorm stay core-local (h kept fp32 in SBUF).
Edge scores s_e (all 8 layers) are precomputed once on device; self-loop
scores are folded into the same score-gather via extra rows of se_all.
"""
import os
import sys
import numpy as np

sys.path.insert(0, "/opt/trn_rl_repo")

GNN_LAYERS = int(os.environ.get("GNN_LAYERS", "8"))

N = 10000
E = 160000
G = 64
D = 512
H = 8
C = 64
L = 8
DE = 256
NCORES = 8
TPC = 10                 # dst-tiles per core
NPC = TPC * 128          # node slots per core
N_PAD = NCORES * NPC
T_FIX = 17               # edge-tiles per dst-tile (final grid)
HALVES = (9, 8)          # edge-tiles per gather call
SPT = T_FIX * 128        # edge slots per dst-tile
ET = TPC * T_FIX
SE_T = 16                # edge-tiles per dst-tile in encoder grid
SE_SPT = SE_T * 128
SE_REAL = TPC * SE_SPT   # real-edge rows of se_all
LOOP_BASE = SE_REAL      # 1280 self-loop rows
ZERO_ROW = SE_REAL + NPC # 128 zero rows start here
SE_ALL_ROWS = ZERO_ROW + 128
REC = 640                # bf16 record: [xh 512 | s_src 8 | pad 120]
NEG = 0.2

_CACHE = {}


# ---------------- host-side prep (integer index work only) ----------------
def _host_prep(edge_index, batch):
    src0 = edge_index[0].astype(np.int64)
    dst0 = edge_index[1].astype(np.int64)
    cnt = np.bincount(dst0, minlength=N)
    inv_cnt = (1.0 / np.maximum(cnt, 1)).astype(np.float32)

    perm_slot = np.full(N, -1, np.int64)
    tile_members = {}
    for core in range(NCORES):
        nodes = np.arange(core * 1250, (core + 1) * 1250)
        deg = cnt[nodes] + 1
        order = np.argsort(-deg, kind="stable")
        loads = np.zeros(TPC, np.int64)
        fill = np.zeros(TPC, np.int64)
        assign = np.empty(1250, np.int64)
        big = np.iinfo(np.int64).max
        for idx in order:
            t = int(np.argmin(np.where(fill < 128, loads, big)))
            assign[idx] = t
            loads[t] += deg[idx]
            fill[t] += 1
        for t in range(TPC):
            members = nodes[assign == t]
            tile_members[(core, t)] = members
            base = core * NPC + t * 128
            perm_slot[members] = base + np.arange(len(members))
        assert loads.max() <= SPT, loads.max()

    slot_node = np.full(N_PAD, -1, np.int64)
    slot_node[perm_slot] = np.arange(N)
    slot_graph = np.full(N_PAD, 999, np.int64)
    valid = slot_node >= 0
    slot_graph[valid] = batch[slot_node[valid]]

    dst_slot_all = perm_slot[dst0]
    dst_core = dst_slot_all // NPC
    dst_tile = (dst_slot_all % NPC) // 128

    ng = NCORES * TPC * SPT
    g_src = np.zeros(ng, np.int64)
    g_dl = np.full(ng, 999, np.int64)
    g_se = np.full(ng, ZERO_ROW, np.int64)

    se_dl = np.full((NCORES, SE_REAL), 999, np.int64)
    se_edge = np.full((NCORES, SE_REAL), -1, np.int64)

    for core in range(NCORES):
        for t in range(TPC):
            sel = np.where((dst_core == core) & (dst_tile == t))[0]
            order = np.argsort(dst_slot_all[sel], kind="stable")
            sel = sel[order]
            n_real = len(sel)
            members = tile_members[(core, t)]
            n_loop = len(members)
            assert n_real <= SE_SPT, n_real
            assert n_real + n_loop <= SPT

            base = (core * TPC + t) * SPT
            g_src[base:base + n_real] = perm_slot[src0[sel]]
            g_dl[base:base + n_real] = dst_slot_all[sel] % 128
            g_se[base:base + n_real] = t * SE_SPT + np.arange(n_real)
            lo = base + n_real
            ms = perm_slot[members]
            g_src[lo:lo + n_loop] = ms
            g_dl[lo:lo + n_loop] = ms % 128
            g_se[lo:lo + n_loop] = LOOP_BASE + t * 128 + (ms % 128)

            sbase = t * SE_SPT
            se_dl[core, sbase:sbase + n_real] = dst_slot_all[sel] % 128
            se_edge[core, sbase:sbase + n_real] = sel

    return dict(perm_slot=perm_slot, slot_node=slot_node, slot_graph=slot_graph,
                inv_cnt=inv_cnt, g_src=g_src, g_dl=g_dl, g_se=g_se,
                se_dl=se_dl, se_edge=se_edge)


def _wrap_idx(flat):
    n = len(flat)
    w = np.asarray(flat, np.int16).reshape(n // 16, 16).T
    return np.tile(w, (8, 1))


def _grid_cols(arr, dtype):
    a = np.asarray(arr).reshape(-1, 128).T
    return np.ascontiguousarray(a).astype(dtype)


def _halves_wrap(vals_for_tile):
    """Wrap one dst-tile's SPT index values into the (9,8)-half layout."""
    out = []
    off = 0
    for gh in HALVES:
        out.append(_wrap_idx(vals_for_tile[off * 128:(off + gh) * 128]))
        off += gh
    return np.concatenate(out, axis=1)


def _build_inputs(inp):
    edge_index = np.asarray(inp["edge_index"])
    batch = np.asarray(inp["batch"])
    prep = _host_prep(edge_index, batch)

    import ml_dtypes
    bf16 = ml_dtypes.bfloat16

    x = np.asarray(inp["x"], np.float32)
    ef = np.asarray(inp["edge_attr"], np.float32)[:, 1:3]
    x_perm = np.zeros((N_PAD, 4), np.float32)
    x_perm[prep["perm_slot"]] = x

    At_src = np.zeros((L, D, H), np.float32)
    At_dst = np.zeros((L, D, H), np.float32)
    At_e = np.zeros((L, D, H), np.float32)
    for l in range(L):
        for h in range(H):
            At_src[l, h * C:(h + 1) * C, h] = np.asarray(inp["a_src"])[l, h]
            At_dst[l, h * C:(h + 1) * C, h] = np.asarray(inp["a_dst"])[l, h]
            At_e[l, h * C:(h + 1) * C, h] = np.asarray(inp["a_e"])[l, h]
    At_sd = np.concatenate([At_src, At_dst], axis=2)

    Wl = np.asarray(inp["Wl"], np.float32)
    WlT = np.ascontiguousarray(np.transpose(Wl, (0, 2, 1)))
    WleT = np.ascontiguousarray(np.transpose(np.asarray(inp["Wle"], np.float32),
                                             (0, 2, 1)))

    shared = {
        "wn": np.asarray(inp["Wn"], np.float32),
        "bn": np.asarray(inp["bn"], np.float32)[None, :],
        "wee": np.asarray(inp["Wee"], np.float32).astype(bf16),
        "bee": np.asarray(inp["bee"], np.float32)[None, :],
        "wleT": WleT.reshape(L * D, DE),
        "at_e": At_e.reshape(L * D, H),
        "at_sd": At_sd.reshape(L * D, 16),
        "wl": Wl.reshape(L * D, D).astype(bf16),
        "wlT": WlT.reshape(L * D, D),
        "bl": np.asarray(inp["bl"], np.float32),
        "wp": np.asarray(inp["Wp"], np.float32),
        "bp": np.asarray(inp["bp"], np.float32)[None, :],
        "wg1": np.asarray(inp["Wg1"], np.float32),
        "bg1": np.asarray(inp["bg1"], np.float32)[None, :],
        "wg2": np.asarray(inp["Wg2"], np.float32),
        "bg2": np.asarray(inp["bg2"], np.float32)[None, :],
        "wf1": np.asarray(inp["Wf1"], np.float32),
        "bf1": np.asarray(inp["bf1"], np.float32)[None, :],
        "wf2": np.asarray(inp["Wf2"], np.float32),
        "bf2": np.asarray(inp["bf2"], np.float32)[None, :],
        "nAT": np.asarray(inp["nA"], np.float32),
        "nBT": np.asarray(inp["nB"], np.float32),
        "sysT": np.asarray(inp["system_size"], np.float32),
    }

    in_maps = []
    for core in range(NCORES):
        lo = core * NPC
        gsl = slice(core * TPC * SPT, (core + 1) * TPC * SPT)
        gs = prep["g_src"][gsl]
        gse = prep["g_se"][gsl]
        gidx = np.concatenate(
            [_halves_wrap(gs[t * SPT:(t + 1) * SPT]) for t in range(TPC)],
            axis=1)
        seidx = np.concatenate(
            [_halves_wrap(gse[t * SPT:(t + 1) * SPT]) for t in range(TPC)],
            axis=1)
        efc = np.zeros((SE_REAL, 2), np.float32)
        rows = prep["se_edge"][core]
        v = rows >= 0
        efc[v] = ef[rows[v]]
        ic = np.zeros((128, TPC), np.float32)
        gid = np.full((128, TPC), 999.0, np.float32)
        for t in range(TPC):
            slots = lo + t * 128 + np.arange(128)
            nodes = prep["slot_node"][slots]
            ok = nodes >= 0
            ic[ok, t] = prep["inv_cnt"][nodes[ok]]
            gid[:, t] = prep["slot_graph"][slots]
        m = dict(shared)
        m.update({
            "xT": np.ascontiguousarray(x_perm[lo:lo + NPC].T),
            "efT": np.ascontiguousarray(efc.T).astype(bf16),
            "gidx": gidx.astype(np.int16),
            "seidx": seidx.astype(np.int16),
            "dstloc": _grid_cols(prep["g_dl"][gsl], np.float32),
            "sedl": _grid_cols(prep["se_dl"][core], np.float32),
            "invcnt": ic,
            "graphid": gid,
        })
        in_maps.append(m)
    return in_maps


# ---------------- bass program ----------------
def _build_program():
    import contextlib
    import concourse.bass as bass
    import concourse.bacc as bacc
    import concourse.tile as tile
    import concourse.mybir as mybir
    from concourse.masks import make_identity

    dt = mybir.dt
    AF = mybir.ActivationFunctionType
    OP = mybir.AluOpType

    nc = bacc.Bacc("TRN2", target_bir_lowering=False, debug=False,
                   num_devices=NCORES)

    def din(name, shape, dtype=dt.float32):
        return nc.dram_tensor(name, shape, dtype, kind="ExternalInput")

    IDXW = sum(HALVES) * 8  # int16 idx cols per dst tile (wrapped)

    xT = din("xT", [4, NPC])
    efT = din("efT", [2, SE_REAL], dt.bfloat16)
    gidx = din("gidx", [128, TPC * IDXW], dt.int16)
    seidx = din("seidx", [128, TPC * IDXW], dt.int16)
    dstloc = din("dstloc", [128, ET])
    sedl = din("sedl", [128, TPC * SE_T])
    invcnt = din("invcnt", [128, TPC])
    graphid = din("graphid", [128, TPC])
    wn = din("wn", [4, D]); bn = din("bn", [1, D])
    wee = din("wee", [2, DE], dt.bfloat16); bee = din("bee", [1, DE])
    wleT = din("wleT", [L * D, DE])
    at_e = din("at_e", [L * D, H])
    at_sd = din("at_sd", [L * D, 16])
    wl = din("wl", [L * D, D], dt.bfloat16)
    wlT = din("wlT", [L * D, D])
    bl = din("bl", [L, D])
    wp = din("wp", [D, D]); bp = din("bp", [1, D])
    wg1 = din("wg1", [2, DE]); bg1 = din("bg1", [1, DE])
    wg2 = din("wg2", [DE, DE]); bg2 = din("bg2", [1, DE])
    wf1 = din("wf1", [D + DE, DE]); bf1 = din("bf1", [1, DE])
    wf2 = din("wf2", [DE, 1]); bf2 = din("bf2", [1, 1])
    nAT = din("nAT", [G, 1]); nBT = din("nBT", [G, 1]); sysT = din("sysT", [G, 1])

    out_t = nc.dram_tensor("out", [G, 1], dt.float32, kind="ExternalOutput")
    RG = [list(range(NCORES))]

    with tile.TileContext(nc) as tc:
        stack = contextlib.ExitStack()
        cst = stack.enter_context(tc.tile_pool(name="cst", bufs=1))
        res = stack.enter_context(tc.tile_pool(name="res", bufs=1))
        wk = stack.enter_context(tc.tile_pool(name="wk", bufs=2))
        gat = stack.enter_context(tc.tile_pool(name="gat", bufs=2))
        ps = stack.enter_context(tc.tile_pool(name="ps", bufs=1, space="PSUM"))
        dram = stack.enter_context(tc.tile_pool(name="dram", bufs=1, space="DRAM"))

        R_bufs = [dram.tile([N_PAD, REC], dt.bfloat16, addr_space="Shared",
                            name=f"R_{i}") for i in range(L)]
        ag_in = dram.tile([NPC, REC], dt.bfloat16)
        se_all = dram.tile([SE_ALL_ROWS, 64], dt.float32)
        selT_dram = dram.tile([NPC, SPT], dt.bfloat16)
        SEL_dram = dram.tile([NPC, SPT], dt.bfloat16)
        pool_in = dram.tile([G, D], dt.float32)
        pool_out = dram.tile([G, D], dt.float32, addr_space="Shared")

        def load_kxn(rows_ap, nchunk, ncols, name, pool=wk, tag=None,
                     dtype=dt.float32):
            t = pool.tile([128, nchunk * ncols], dtype, name=name,
                          tag=tag or name)
            for kc in range(nchunk):
                nc.sync.dma_start(
                    out=t[:, kc * ncols:(kc + 1) * ncols],
                    in_=rows_ap[kc * 128:(kc + 1) * 128, :])
            return t

        def ldma(src_ap, shape, name, pool=cst, dtype=dt.float32, tag=None,
                 bufs=None):
            t = pool.tile(list(shape), dtype, name=name, tag=tag or name,
                          bufs=bufs)
            nc.sync.dma_start(out=t[:], in_=src_ap)
            return t

        def rep_row(row_ap, p, f, name, pool=cst, tag=None, bufs=None):
            t = pool.tile([p, f], dt.float32, name=name, tag=tag or name,
                          bufs=bufs)
            nc.sync.dma_start(out=t[:], in_=row_ap.to_broadcast((p, f)))
            return t

        # constants
        ident_g = cst.tile([128, 128], dt.float32)
        make_identity(nc, ident_g[:])
        ident = cst.tile([128, 128], dt.float32)
        nc.vector.tensor_copy(ident[:], ident_g[:])
        ident_b = cst.tile([128, 128], dt.bfloat16)
        nc.vector.tensor_copy(ident_b[:], ident_g[:])
        iota_i = cst.tile([128, 128], dt.int32)
        nc.gpsimd.iota(iota_i[:], pattern=[[1, 128]], base=0, channel_multiplier=0)
        iotaF = cst.tile([128, 128], dt.float32)
        nc.vector.tensor_copy(iotaF[:], iota_i[:])
        iota64_i = cst.tile([128, G], dt.int32)
        nc.gpsimd.iota(iota64_i[:], pattern=[[1, G]], base=0, channel_multiplier=0)
        iota64 = cst.tile([128, G], dt.float32)
        nc.vector.tensor_copy(iota64[:], iota64_i[:])

        wn_sb = ldma(wn[:], (4, D), "wn_sb")
        wee_sb = ldma(wee[:], (2, DE), "wee_sb", dtype=dt.bfloat16)
        dstloc_sb = ldma(dstloc[:], (128, ET), "dstloc_sb")
        sedl_sb = ldma(sedl[:], (128, TPC * SE_T), "sedl_sb")
        invcnt_sb = ldma(invcnt[:], (128, TPC), "invcnt_sb")
        graphid_sb = ldma(graphid[:], (128, TPC), "graphid_sb")
        gidx_sb = ldma(gidx[:], (128, TPC * IDXW), "gidx_sb", dtype=dt.int16)
        seidx_sb = ldma(seidx[:], (128, TPC * IDXW), "seidx_sb", dtype=dt.int16)
        bn_rep = rep_row(bn[:], 128, D, "bn_rep")
        bee_rep = rep_row(bee[:], 128, DE, "bee_rep")
        xT_sb = ldma(xT[:], (4, NPC), "xT_sb")

        h_my = res.tile([128, TPC * D], dt.float32)
        s_my = res.tile([128, TPC * 16], dt.bfloat16)
        s_e_sb = res.tile([128, ET * 64], dt.float16)

        def psum(shape, tag, bufs, name, dtype=dt.float32):
            return ps.tile(list(shape), dtype, space="PSUM", name=name,
                           tag=tag, bufs=bufs)

        def ln_store(src_ap, dst_ap, F, bias_rep=None, do_elu=True,
                     residual_ap=None, P=128):
            x1 = wk.tile([P, F], dt.float32, name="ln_x1", tag="ln_x1")
            if bias_rep is not None:
                nc.vector.tensor_tensor(out=x1[:], in0=src_ap,
                                        in1=bias_rep[:P, :F], op=OP.add)
            else:
                nc.vector.tensor_copy(x1[:], src_ap)
            sums = wk.tile([P, 1], dt.float32, name="ln_sum", tag="ln_sum")
            sc = wk.tile([P, F], dt.float32, name="ln_sc", tag="ln_sc")
            nc.scalar.activation(out=sc[:], in_=x1[:], func=AF.Copy,
                                 accum_out=sums[:])
            mean = wk.tile([P, 1], dt.float32, name="ln_mean", tag="ln_mean")
            nc.vector.tensor_scalar_mul(mean[:], sums[:], 1.0 / F)
            nc.vector.tensor_scalar(out=x1[:], in0=x1[:], scalar1=mean[:],
                                    scalar2=None, op0=OP.subtract)
            sq = wk.tile([P, 1], dt.float32, name="ln_sq", tag="ln_sq")
            nc.scalar.activation(out=sc[:], in_=x1[:], func=AF.Square,
                                 accum_out=sq[:])
            # rstd = exp(-0.5*ln(var+eps)); Ln/Exp live in the same act table
            # as Copy/Square so the scalar engine never reloads tables.
            var = wk.tile([P, 1], dt.float32, name="ln_var", tag="ln_var")
            nc.vector.tensor_scalar(out=var[:], in0=sq[:], scalar1=1.0 / F,
                                    scalar2=1e-5, op0=OP.mult, op1=OP.add)
            lnv = wk.tile([P, 1], dt.float32, name="ln_lnv", tag="ln_lnv")
            nc.scalar.activation(out=lnv[:], in_=var[:], func=AF.Ln)
            rstd = wk.tile([P, 1], dt.float32, name="ln_rstd", tag="ln_rstd")
            nc.scalar.activation(out=rstd[:], in_=lnv[:], func=AF.Exp,
                                 scale=-0.5)
            nc.vector.tensor_scalar(out=x1[:], in0=x1[:], scalar1=rstd[:],
                                    scalar2=None, op0=OP.mult)
            if do_elu:
                tmin = wk.tile([P, F], dt.float32, name="ln_tm", tag="ln_sc")
                nc.vector.tensor_scalar_min(tmin[:], x1[:], 0.0)
                nc.scalar.activation(out=tmin[:], in_=tmin[:], func=AF.Exp)
                nc.vector.tensor_scalar_max(x1[:], x1[:], 0.0)
                nc.vector.tensor_tensor(out=x1[:], in0=x1[:], in1=tmin[:],
                                        op=OP.add)
                nc.vector.tensor_scalar_add(x1[:], x1[:], -1.0)
            if residual_ap is not None:
                nc.vector.tensor_tensor(out=dst_ap, in0=x1[:], in1=residual_ap,
                                        op=OP.add)
            else:
                nc.vector.tensor_copy(dst_ap, x1[:])

        def transpose_chunks(src_ap, nchunk, rows=128, out_dtype=dt.float32):
            dst = wk.tile([128, nchunk * rows], out_dtype, name="trT", tag="trT")
            src_bf = src_ap.dtype == dt.bfloat16
            idn = ident_b if src_bf else ident
            for ci in range(nchunk):
                pt = psum([128, rows], "tr", 2, "tr_ps",
                          dtype=src_ap.dtype if src_bf else dt.float32)
                nc.tensor.transpose(out=pt[:],
                                    in_=src_ap[:, ci * 128:(ci + 1) * 128],
                                    identity=idn[:rows, :rows])
                nc.vector.tensor_copy(dst[:, ci * rows:(ci + 1) * rows], pt[:])
            return dst

        # ---------- setup: Wes / Wsd ----------
        wes_b = cst.tile([128, 2 * 64], dt.bfloat16)
        for l in range(L):
            for m in range(2):
                pt = psum([128, H], "sacc", 1, "wes_ps")
                for k in range(4):
                    lhs = ldma(wleT[l * D + k * 128:l * D + (k + 1) * 128,
                                    m * 128:(m + 1) * 128], (128, 128), "wleT_c",
                               pool=wk, tag="wleT_c")
                    rhs = ldma(at_e[l * D + k * 128:l * D + (k + 1) * 128, :],
                               (128, H), "ate_c", pool=wk, tag="ate_c")
                    nc.tensor.matmul(pt[:], lhs[:], rhs[:], start=(k == 0),
                                     stop=(k == 3))
                nc.vector.tensor_copy(
                    wes_b[:, m * 64 + l * 8:m * 64 + (l + 1) * 8], pt[:])

        wsd_b = cst.tile([128, 4 * L * 16], dt.bfloat16)
        for l in range(L):
            for kc in range(4):
                pt = psum([128, 16], "tr", 2, "wsd_ps")
                for oc in range(4):
                    lhs = ldma(wlT[l * D + oc * 128:l * D + (oc + 1) * 128,
                                   kc * 128:(kc + 1) * 128], (128, 128), "wlT_c",
                               pool=wk, tag="wleT_c")
                    rhs = ldma(at_sd[l * D + oc * 128:l * D + (oc + 1) * 128, :],
                               (128, 16), "atsd_c", pool=wk, tag="ate_c")
                    nc.tensor.matmul(pt[:], lhs[:], rhs[:], start=(oc == 0),
                                     stop=(oc == 3))
                nc.vector.tensor_copy(
                    wsd_b[:, (kc * L + l) * 16:(kc * L + l + 1) * 16], pt[:])

        # ---------- setup: h0 ----------
        for t in range(TPC):
            pt = psum([128, D], "gemm", 2, "h0_ps")
            nc.tensor.matmul(pt[:], xT_sb[:, t * 128:(t + 1) * 128], wn_sb[:],
                             start=True, stop=True)
            ln_store(pt[:], h_my[:, t * D:(t + 1) * D], D, bias_rep=bn_rep,
                     do_elu=True)

        # ---------- setup: static selection matrices -> DRAM ----------
        for t in range(TPC):
            selT_f = wk.tile([128, SPT], dt.float32, name="selT_f",
                             tag="selT_f", bufs=1)
            for j in range(T_FIX):
                tj = t * T_FIX + j
                nc.vector.tensor_tensor(
                    out=selT_f[:, j * 128:(j + 1) * 128],
                    in0=dstloc_sb[:, tj:tj + 1].to_broadcast((128, 128)),
                    in1=iotaF[:], op=OP.is_equal)
            selT_b = wk.tile([128, SPT], dt.bfloat16, name="selT_b",
                             tag="selT_b", bufs=1)
            nc.vector.tensor_copy(selT_b[:], selT_f[:])
            nc.sync.dma_start(out=selT_dram[t * 128:(t + 1) * 128, :],
                              in_=selT_b[:])
            SEL_b = wk.tile([128, SPT], dt.bfloat16, name="SEL_b", tag="SEL_b",
                            bufs=1)
            for j in range(T_FIX):
                pt = psum([128, 128], "tr", 2, "selTr_ps")
                nc.tensor.transpose(out=pt[:],
                                    in_=selT_f[:, j * 128:(j + 1) * 128],
                                    identity=ident[:])
                nc.vector.tensor_copy(SEL_b[:, j * 128:(j + 1) * 128], pt[:])
            nc.sync.dma_start(out=SEL_dram[t * 128:(t + 1) * 128, :],
                              in_=SEL_b[:])

        # ---------- setup: s_e encoder over real-edge grid ----------
        zrow = wk.tile([128, 64], dt.float32, name="zrow", tag="sev")
        nc.vector.memset(zrow[:], 0.0)
        nc.sync.dma_start(out=se_all[ZERO_ROW:ZERO_ROW + 128, :], in_=zrow[:])
        for t in range(TPC):
            eft_all = gat.tile([2, SE_SPT], dt.bfloat16, name="eft", tag="eft",
                               bufs=1)
            nc.sync.dma_start(out=eft_all[:],
                              in_=efT[:, t * SE_SPT:(t + 1) * SE_SPT])
            lps = psum([128, 64], "agg", 2, "loop_ps")
            for k in range(SE_T):
                i = t * SE_T + k
                ept = psum([128, DE], "gemm", 2, "ee_ps")
                nc.tensor.matmul(ept[:], eft_all[:, k * 128:(k + 1) * 128],
                                 wee_sb[:], start=True, stop=True)
                ee = wk.tile([128, DE], dt.bfloat16, name="ee_sb", tag="ee_sb")
                ln_store(ept[:], ee[:], DE, bias_rep=bee_rep, do_elu=True)
                eeT = transpose_chunks(ee[:], 2, out_dtype=dt.bfloat16)
                spt = psum([128, 64], "eps", 1, "se_ps")
                for mc in range(2):
                    nc.tensor.matmul(spt[:], eeT[:, mc * 128:(mc + 1) * 128],
                                     wes_b[:, mc * 64:(mc + 1) * 64],
                                     start=(mc == 0), stop=(mc == 1))
                sev = wk.tile([128, 64], dt.float32, name="sev", tag="sev")
                nc.vector.tensor_copy(sev[:], spt[:])
                nc.sync.dma_start(out=se_all[i * 128:(i + 1) * 128, :], in_=sev[:])
                selT = wk.tile([128, 128], dt.float32, name="selTse", tag="selT")
                nc.vector.tensor_tensor(
                    out=selT[:], in0=sedl_sb[:, i:i + 1].to_broadcast((128, 128)),
                    in1=iotaF[:], op=OP.is_equal)
                nc.tensor.matmul(lps[:], selT[:], sev[:], start=(k == 0),
                                 stop=(k == SE_T - 1))
            lse = wk.tile([128, 64], dt.float32, name="lse", tag="sev")
            nc.vector.tensor_tensor(
                out=lse[:], in0=lps[:],
                in1=invcnt_sb[:, t:t + 1].to_broadcast((128, 64)), op=OP.mult)
            nc.sync.dma_start(
                out=se_all[LOOP_BASE + t * 128:LOOP_BASE + (t + 1) * 128, :],
                in_=lse[:])

        # ---------- setup: gather s_e into final grid ----------
        for t in range(TPC):
            off = 0
            for hf, GHh in enumerate(HALVES):
                sg = gat.tile([128, HALVES[0] * 64], dt.float32, name="seg",
                              tag="seg")
                nc.gpsimd.dma_gather(
                    out_ap=sg[:, :GHh * 64].rearrange("p (t e) -> p t e", e=64),
                    in_ap=se_all[:],
                    idxs_ap=seidx_sb[:, t * IDXW + off * 8:
                                     t * IDXW + (off + GHh) * 8],
                    num_idxs=GHh * 128, num_idxs_reg=GHh * 128, elem_size=64,
                    single_packet=False)
                nc.vector.tensor_copy(
                    s_e_sb[:, (t * T_FIX + off) * 64:
                           (t * T_FIX + off + GHh) * 64],
                    sg[:, :GHh * 64])
                off += GHh

        # ---------- record emit (xh | s_src) for layer lx from current h ----
        def emit_record(t, lx, wl_tile):
            hT_f = transpose_chunks(h_my[:, t * D:(t + 1) * D], 4,
                                    out_dtype=dt.bfloat16)
            xp = psum([128, D], "gemm", 2, "xh_ps")
            for k in range(4):
                nc.tensor.matmul(xp[:], hT_f[:, k * 128:(k + 1) * 128],
                                 wl_tile[:, k * D:(k + 1) * D],
                                 start=(k == 0), stop=(k == 3))
            xh_sb = wk.tile([128, D], dt.bfloat16, name="xh_sb", tag="xh_sb")
            nc.vector.tensor_copy(xh_sb[:], xp[:])
            s16 = psum([128, 16], "tr", 2, "s16_ps")
            for k in range(4):
                nc.tensor.matmul(s16[:], hT_f[:, k * 128:(k + 1) * 128],
                                 wsd_b[:, (k * L + lx) * 16:(k * L + lx + 1) * 16],
                                 start=(k == 0), stop=(k == 3))
            nc.vector.tensor_copy(s_my[:, t * 16:(t + 1) * 16], s16[:])
            nc.sync.dma_start(out=ag_in[t * 128:(t + 1) * 128, 0:D], in_=xh_sb[:])
            nc.sync.dma_start(out=ag_in[t * 128:(t + 1) * 128, D:D + 8],
                              in_=s_my[:, t * 16:t * 16 + 8])

        wl_cur = load_kxn(wl[0:D, :], 4, D, "wl_sb0", tag="wl_sb",
                          dtype=dt.bfloat16)
        for t in range(TPC):
            emit_record(t, 0, wl_cur)
        nc.gpsimd.collective_compute("AllGather", OP.bypass, replica_groups=RG,
                                     ins=[ag_in.opt()], outs=[R_bufs[0].opt()])

        # ---------- layers ----------
        pool_holder = [None]
        LL = GNN_LAYERS
        for l in range(LL):
            R_cur = R_bufs[l]
            R_next = R_bufs[l + 1] if l < L - 1 else None
            wl_next = None
            if l < L - 1:
                wl_next = load_kxn(wl[(l + 1) * D:(l + 2) * D, :], 4, D,
                                   f"wl_sb{l + 1}", tag="wl_sb",
                                   dtype=dt.bfloat16)
            bl_rep = rep_row(bl[l:l + 1, :], 128, D, f"bl_rep{l}", pool=wk,
                             tag="bl_rep")
            for t in range(TPC):
                selTs = gat.tile([128, SPT], dt.bfloat16, name="selTs",
                                 tag="selTs")
                nc.sync.dma_start(out=selTs[:],
                                  in_=selT_dram[t * 128:(t + 1) * 128, :])
                SELs = gat.tile([128, SPT], dt.bfloat16, name="SELs",
                                tag="SELs")
                nc.sync.dma_start(out=SELs[:],
                                  in_=SEL_dram[t * 128:(t + 1) * 128, :])
                sacc = psum([128, 8], "sacc", 1, "sacc_ps")
                eps_ps = psum([128, T_FIX * H], "eps", 1, "eps_ps")
                agg_ps = psum([128, D], "agg", 2, "agg_ps")
                al_f = wk.tile([128, T_FIX * H], dt.float32, name="al",
                               tag="al")
                al2 = wk.tile([128, T_FIX * H], dt.float32, name="al2",
                              tag="al2")
                pb_b = wk.tile([128, T_FIX * H], dt.bfloat16, name="pb",
                               tag="pb")
                y_b = wk.tile([128, HALVES[0] * D], dt.bfloat16, name="y",
                              tag="y", bufs=1)
                off = 0
                for hf, GHh in enumerate(HALVES):
                    gt = gat.tile([128, HALVES[0] * REC], dt.bfloat16,
                                  name="gt", tag="gt")
                    nc.gpsimd.dma_gather(
                        out_ap=gt[:, :GHh * REC].rearrange(
                            "p (t e) -> p t e", e=REC),
                        in_ap=R_cur[:],
                        idxs_ap=gidx_sb[:, t * IDXW + off * 8:
                                        t * IDXW + (off + GHh) * 8],
                        num_idxs=GHh * 128, num_idxs_reg=GHh * 128,
                        elem_size=REC, single_packet=False)
                    for j in range(GHh):
                        jj = off + j
                        nc.tensor.matmul(
                            eps_ps[:, jj * 8:(jj + 1) * 8],
                            SELs[:, jj * 128:(jj + 1) * 128],
                            s_my[:, t * 16 + 8:(t + 1) * 16],
                            start=True, stop=True)
                    cs = slice(off * 8, (off + GHh) * 8)
                    g3 = gt[:, :GHh * REC].rearrange("p (g r) -> p g r", r=REC)
                    nc.vector.tensor_tensor(
                        out=al_f[:, cs].rearrange("p (g h) -> p g h", h=H),
                        in0=g3[:, :, D:D + 8],
                        in1=s_e_sb[:, (t * T_FIX) * 64:(t + 1) * T_FIX * 64]
                        .rearrange("p (g s) -> p g s", s=64)
                        [:, off:off + GHh, l * 8:(l + 1) * 8],
                        op=OP.add)
                    nc.vector.tensor_tensor(out=al_f[:, cs], in0=al_f[:, cs],
                                            in1=eps_ps[:, cs], op=OP.add)
                    nc.vector.tensor_scalar_mul(al2[:, cs], al_f[:, cs], NEG)
                    nc.vector.tensor_tensor(out=al_f[:, cs], in0=al_f[:, cs],
                                            in1=al2[:, cs], op=OP.max)
                    nc.scalar.activation(out=pb_b[:, cs], in_=al_f[:, cs],
                                         func=AF.Exp)
                    nc.vector.tensor_tensor(
                        out=y_b[:, :GHh * D].rearrange(
                            "p (g h c) -> p g h c", h=H, c=C),
                        in0=g3[:, :, 0:D].rearrange(
                            "p g (h c) -> p g h c", c=C),
                        in1=pb_b[:, cs].rearrange("p (g h) -> p g h", h=H)
                        [:, :, :, None].to_broadcast((128, GHh, H, C)),
                        op=OP.mult)
                    for j in range(GHh):
                        jj = off + j
                        nc.tensor.matmul(sacc[:],
                                         selTs[:, jj * 128:(jj + 1) * 128],
                                         pb_b[:, jj * 8:(jj + 1) * 8],
                                         start=(jj == 0), stop=(jj == T_FIX - 1))
                        nc.tensor.matmul(agg_ps[:],
                                         selTs[:, jj * 128:(jj + 1) * 128],
                                         y_b[:, j * D:(j + 1) * D],
                                         start=(jj == 0), stop=(jj == T_FIX - 1))
                    off += GHh
                # ---- phase B ----
                spl = wk.tile([128, H], dt.float32, name="spl", tag="spl")
                nc.vector.tensor_scalar_add(spl[:], sacc[:], 1e-16)
                rr = wk.tile([128, H], dt.float32, name="rr", tag="rr")
                nc.vector.reciprocal(rr[:], spl[:])
                aggn = wk.tile([128, D], dt.float32, name="aggn", tag="aggn",
                               bufs=1)
                nc.vector.tensor_tensor(
                    out=aggn[:].rearrange("p (h c) -> p h c", c=C),
                    in0=agg_ps[:].rearrange("p (h c) -> p h c", c=C),
                    in1=rr[:][:, :, None].to_broadcast((128, H, C)), op=OP.mult)
                ln_store(aggn[:], h_my[:, t * D:(t + 1) * D], D, bias_rep=bl_rep,
                         do_elu=False, residual_ap=h_my[:, t * D:(t + 1) * D])
                if l == LL - 1:
                    gsel = wk.tile([128, G], dt.float32, name="gsel", tag="gsel")
                    nc.vector.tensor_tensor(
                        out=gsel[:],
                        in0=graphid_sb[:, t:t + 1].to_broadcast((128, G)),
                        in1=iota64[:], op=OP.is_equal)
                    if pool_holder[0] is None:
                        pool_holder[0] = psum([G, D], "gemm", 2, "pool_ps")
                    nc.tensor.matmul(pool_holder[0][:], gsel[:],
                                     h_my[:, t * D:(t + 1) * D],
                                     start=(t == 0), stop=(t == TPC - 1))
                else:
                    emit_record(t, l + 1, wl_next)
            if l < LL - 1:
                nc.gpsimd.collective_compute("AllGather", OP.bypass,
                                             replica_groups=RG,
                                             ins=[ag_in.opt()],
                                             outs=[R_next.opt()])

        # ---------- readout ----------
        pool_sb = wk.tile([G, D], dt.float32, name="pool_sb", tag="pool_sb")
        nc.vector.tensor_copy(pool_sb[:], pool_holder[0][:])
        nc.sync.dma_start(out=pool_in[:], in_=pool_sb[:])
        nc.gpsimd.collective_compute("AllReduce", OP.add, replica_groups=RG,
                                     ins=[pool_in.opt()], outs=[pool_out.opt()])
        hp = ldma(pool_out[:], (G, D), "hp_sb", pool=wk, tag="pool_sb")

        def transpose_to64(src_ap, nchunk):
            dst = wk.tile([128, nchunk * G], dt.float32, name="t64", tag="t64")
            for ci in range(nchunk):
                pt = psum([128, G], "tr", 2, "t64_ps")
                nc.tensor.transpose(out=pt[:],
                                    in_=src_ap[:, ci * 128:(ci + 1) * 128],
                                    identity=ident[:G, :G])
                nc.vector.tensor_copy(dst[:, ci * G:(ci + 1) * G], pt[:])
            return dst

        wp_sb = load_kxn(wp[:], 4, D, "wp_sb", tag="wp_sb")
        bp_rep = rep_row(bp[:], G, D, "bp_rep", pool=wk, tag="bl_rep")
        hpT = transpose_to64(hp[:], 4)
        hr_ps = psum([G, D], "gemm", 2, "hr_ps")
        for k in range(4):
            nc.tensor.matmul(hr_ps[:], hpT[:, k * G:(k + 1) * G],
                             wp_sb[:, k * D:(k + 1) * D], start=(k == 0),
                             stop=(k == 3))
        h_r = wk.tile([G, D], dt.float32, name="h_r", tag="h_r")
        ln_store(hr_ps[:], h_r[:], D, bias_rep=bp_rep, do_elu=True, P=G)

        nA_sb = ldma(nAT[:], (G, 1), "nA_sb")
        nB_sb = ldma(nBT[:], (G, 1), "nB_sb")
        sys_sb = ldma(sysT[:], (G, 1), "sys_sb")
        invg = wk.tile([G, 1], dt.float32, name="invg", tag="invg")
        nc.vector.tensor_scalar_add(invg[:], sys_sb[:], 1e-10)
        nc.vector.reciprocal(invg[:], invg[:])
        gf = wk.tile([G, 2], dt.float32, name="gf", tag="gf")
        nc.vector.tensor_tensor(out=gf[:, 0:1], in0=nA_sb[:], in1=invg[:],
                                op=OP.mult)
        nc.vector.tensor_tensor(out=gf[:, 1:2], in0=nB_sb[:], in1=invg[:],
                                op=OP.mult)
        gft_ps = psum([2, G], "tr", 2, "gft_ps")
        nc.tensor.transpose(out=gft_ps[:], in_=gf[:], identity=ident[:G, :G])
        gfT = wk.tile([2, G], dt.float32, name="gfT", tag="gfT")
        nc.vector.tensor_copy(gfT[:], gft_ps[:])

        wg1_sb = ldma(wg1[:], (2, DE), "wg1_sb")
        bg1_rep = rep_row(bg1[:], G, DE, "bg1_rep", pool=wk, tag="bl_rep")
        g1_ps = psum([G, DE], "gemm", 2, "g1_ps")
        nc.tensor.matmul(g1_ps[:], gfT[:], wg1_sb[:], start=True, stop=True)
        gm1 = wk.tile([G, DE], dt.float32, name="gm1", tag="gm1")
        ln_store(g1_ps[:], gm1[:], DE, bias_rep=bg1_rep, do_elu=True, P=G)

        wg2_sb = load_kxn(wg2[:], 2, DE, "wg2_sb", tag="wp_sb")
        bg2_rep = rep_row(bg2[:], G, DE, "bg2_rep", pool=wk, tag="bl_rep")
        gm1T = transpose_to64(gm1[:], 2)
        g2_ps = psum([G, DE], "gemm", 2, "g2_ps")
        for k in range(2):
            nc.tensor.matmul(g2_ps[:], gm1T[:, k * G:(k + 1) * G],
                             wg2_sb[:, k * DE:(k + 1) * DE], start=(k == 0),
                             stop=(k == 1))
        gm2 = wk.tile([G, DE], dt.float32, name="gm2", tag="gm2")
        ln_store(g2_ps[:], gm2[:], DE, bias_rep=bg2_rep, do_elu=True, P=G)

        wf1_sb = load_kxn(wf1[:], 6, DE, "wf1_sb", tag="wp_sb")
        bf1_rep = rep_row(bf1[:], G, DE, "bf1_rep", pool=wk, tag="bl_rep")
        hrT = transpose_to64(h_r[:], 4)
        gm2T = wk.tile([128, 2 * G], dt.float32, name="gm2T", tag="gm2T")
        for ci in range(2):
            pt = psum([128, G], "tr", 2, "gm2t_ps")
            nc.tensor.transpose(out=pt[:], in_=gm2[:, ci * 128:(ci + 1) * 128],
                                identity=ident[:G, :G])
            nc.vector.tensor_copy(gm2T[:, ci * G:(ci + 1) * G], pt[:])
        f1_ps = psum([G, DE], "gemm", 2, "f1_ps")
        for k in range(4):
            nc.tensor.matmul(f1_ps[:], hrT[:, k * G:(k + 1) * G],
                             wf1_sb[:, k * DE:(k + 1) * DE], start=(k == 0),
                             stop=False)
        for k in range(2):
            nc.tensor.matmul(f1_ps[:], gm2T[:, k * G:(k + 1) * G],
                             wf1_sb[:, (4 + k) * DE:(5 + k) * DE], start=False,
                             stop=(k == 1))
        f1 = wk.tile([G, DE], dt.float32, name="f1", tag="f1")
        ln_store(f1_ps[:], f1[:], DE, bias_rep=bf1_rep, do_elu=True, P=G)

        wf2_sb = load_kxn(wf2[:], 2, 1, "wf2_sb", tag="wf2_sb")
        bf2_rep = rep_row(bf2[:], G, 1, "bf2_rep", pool=wk, tag="bf2_rep")
        f1T = transpose_to64(f1[:], 2)
        o_ps = psum([G, 1], "eps", 1, "o_ps")
        for k in range(2):
            nc.tensor.matmul(o_ps[:], f1T[:, k * G:(k + 1) * G],
                             wf2_sb[:, k:k + 1], start=(k == 0), stop=(k == 1))
        ovec = wk.tile([G, 1], dt.float32, name="ovec", tag="ovec")
        nc.vector.tensor_tensor(out=ovec[:], in0=o_ps[:], in1=bf2_rep[:],
                                op=OP.add)
        nc.sync.dma_start(out=out_t[:], in_=ovec[:])

        stack.close()

    nc.compile()
    return nc


def kernel(**inputs) -> np.ndarray:
    from concourse.bass_utils import run_bass_kernel_spmd
    if "nc" not in _CACHE:
        _CACHE["nc"] = _build_program()
    nc = _CACHE["nc"]
    in_maps = _build_inputs(inputs)
    res = run_bass_kernel_spmd(nc, in_maps, core_ids=list(range(NCORES)))
    out = res.results[0]["out"]
    return np.asarray(out).reshape(G).astype(np.float32)
